# revision 1
# baseline (speedup 1.0000x reference)
"""CycleMatcher (mutual-nearest-neighbor descriptor matching) on 8 trn2 cores.

Problem: B=4 pairs of L2-normalized descriptor sets d0,d1 [8192, 64].
dist = sqrt2*sqrt(clip(1 - d0@d1.T, 1e-6)) ; row/col argmins; mutual-NN
masking; scatter. dist is monotone-decreasing in sim = d0@d1.T, so argmin
dist == argmax sim (with care for fp32 sqrt rounding ties, resolved on host).

Sharding: 8 cores = 4 batches x 2 orientations. Core (b, 0) computes
S = d0[b] @ d1[b].T row-argmax (n_amin side); core (b, 1) computes
S.T = d1[b] @ d0[b].T row-argmax (m_amin side). Identical device program,
inputs swapped.

Device program per core: for each 128-row strip (64 strips), fp32 matmuls
[64,128]^T @ [64,512] fill PSUM in [128, 2048] groups (4 banks, double
buffered); ScalarE drains each group to SBUF; DVE `max` (top-8 values) +
`max_index` (their indices) reduce each SBUF group. Exports per row
4 groups x top-8 (value, local index) candidates. Host merges candidates,
resolves sqrt-rounding ties exactly in reference fp32 semantics
(fp64-refining near-ties), then does the cheap mutual-NN match + scatter
in numpy. Measured device time ~1.17 ms (DVE-bound: 2 passes over 67M
fp32 elements at 1 elem/cycle/lane, 0.96 GHz).
"""

import os
import sys

# Prefer whatever copy PYTHONPATH already provides (the axon sitecustomize
# puts /root/.axon_site/_ro/trn_rl_repo there); append fallbacks so kernel.py
# also works standalone without creating dual module identities.
for _p in ("/root/.axon_site/_ro/trn_rl_repo", "/opt/trn_rl_repo"):
    if _p not in sys.path:
        sys.path.append(_p)

import numpy as np

import concourse.bass as bass
import concourse.mybir as mybir
import concourse.tile as tile
from concourse import bacc
from concourse.bass_utils import run_bass_kernel_spmd

B = 4
M = 8192
N = 8192
D = 64

PART = 128          # rows per strip (psum partitions)
NSTRIP = M // PART  # 64
MMN = 512           # matmul moving free dim (one psum bank, fp32)
GRP = int(os.environ.get("KERNEL_GRP", "2048"))  # psum group / DVE op width
NG = N // GRP       # 4 groups per strip
TOPK = 8            # DVE max/max_index width

# Variant is needed before CAND can be fixed (strip variant exports one
# top-8 per row, group variants export one per [128, GRP] group).
_VARIANT_ENV = os.environ.get("KERNEL_VARIANT", "sbuf")
# candidate groups per row by variant: (n_groups, group_width)
_GROUPS = {"strip": (1, N), "sbuf4k": (2, 2 * GRP)}.get(_VARIANT_ENV, (NG, GRP))
CAND = _GROUPS[0] * TOPK

SQRT_2 = np.float32(1.414213)

# Ablation for differential timing only: 0 = full, 1 = no max_index,
# 2 = no max/max_index (matmuls only). Never set for real runs.
_ABLATE = int(os.environ.get("KERNEL_ABLATE", "0"))
# Variants (KERNEL_VARIANT):
#   psum:  DVE max/max_index read PSUM groups directly (1.54 ms measured)
#   sbuf:  ScalarE drains each PSUM group to SBUF; DVE reduces [128,2048]
#          SBUF groups (1.17 ms — PSUM-sourced DVE ops pay extra access
#          overhead and contend with PE writes; ScalarE is otherwise idle)
#   strip: like sbuf but DVE reduces whole [128,8192] strips in one
#          max + one max_index (measured 3.6 ms - large DVE ops pay
#          duration-proportional DRAIN; do not use)
_VARIANT = _VARIANT_ENV

_prog_cache = {}


def _build_program():
    # KERNEL_REPEATS > 1 repeats the whole compute loop (unrolled);
    # KERNEL_LOOP > 1 wraps it in an on-device For_i (constant program size).
    # Both are only for differential wall-clock timing: axon dispatch
    # overhead dominates a single run, the slope over repeats isolates
    # device time.
    repeats = int(os.environ.get("KERNEL_REPEATS", "1"))
    loops = int(os.environ.get("KERNEL_LOOP", "1"))
    nc = bacc.Bacc("TRN2", target_bir_lowering=False, debug=False)
    f32 = mybir.dt.float32
    u32 = mybir.dt.uint32

    at_d = nc.dram_tensor("at", [D, M], f32, kind="ExternalInput")
    bt_d = nc.dram_tensor("bt", [D, N], f32, kind="ExternalInput")
    vals_d = nc.dram_tensor("vals", [PART, NSTRIP * CAND], f32, kind="ExternalOutput")
    idxs_d = nc.dram_tensor("idxs", [PART, NSTRIP * CAND], u32, kind="ExternalOutput")

    with tile.TileContext(nc) as tc:
        with (
            tc.tile_pool(name="inp", bufs=1) as inp,
            tc.tile_pool(name="outp", bufs=1) as outp,
            tc.tile_pool(name="ps", bufs=2, space="PSUM") as ps,
            tc.tile_pool(
                name="stage",
                bufs={"strip": 2, "sbuf2": 8, "sbuf4k": 3}.get(_VARIANT, 4),
            ) as stage,
        ):
            at = inp.tile([D, M], f32)
            bt = inp.tile([D, N], f32)
            # two different HWDGE queues so the loads overlap
            nc.sync.dma_start(at[:], at_d.ap())
            nc.scalar.dma_start(bt[:], bt_d.ap())

            vals = outp.tile([PART, NSTRIP * CAND], f32)
            idxs = outp.tile([PART, NSTRIP * CAND], u32)

            def body():
                for m in [mm % NSTRIP for mm in range(NSTRIP * repeats)]:
                    lhsT = at[:, m * PART:(m + 1) * PART]  # [64, 128] stationary
                    strip = None
                    if _VARIANT == "strip":
                        strip = stage.tile([PART, N], f32, tag="strip")
                    sts = []
                    for g in range(NG):
                        pt = ps.tile([PART, GRP], f32)
                        for j in range(GRP // MMN):
                            n0 = g * GRP + j * MMN
                            nc.tensor.matmul(
                                pt[:, j * MMN:(j + 1) * MMN],
                                lhsT,
                                bt[:, n0:n0 + MMN],
                                start=True,
                                stop=True,
                            )
                        if _VARIANT == "strip":
                            nc.scalar.copy(strip[:, g * GRP:(g + 1) * GRP], pt[:])
                            continue
                        if _VARIANT == "sbuf4k":
                            # two PSUM groups share one [128, 4096] stage
                            # tile; DVE reduces it in one max + max_index
                            if g % 2 == 0:
                                st4 = stage.tile([PART, 2 * GRP], f32, tag="st4")
                            nc.scalar.copy(
                                st4[:, (g % 2) * GRP:(g % 2 + 1) * GRP], pt[:]
                            )
                            if g % 2 == 1:
                                gg = g // 2
                                c0 = m * CAND + gg * TOPK
                                vs = vals[:, c0:c0 + TOPK]
                                nc.vector.max(out=vs, in_=st4[:])
                                nc.vector.max_index(
                                    out=idxs[:, c0:c0 + TOPK],
                                    in_max=vs,
                                    in_values=st4[:],
                                )
                            continue
                        if _VARIANT == "sbuf2":
                            # drain now; issue all max ops, then all
                            # max_index ops after the group loop so
                            # independent work sits between dependent pairs
                            st = stage.tile([PART, GRP], f32, tag="st2")
                            nc.scalar.copy(st[:], pt[:])
                            sts.append(st)
                            continue
                        c0 = m * CAND + g * TOPK
                        vs = vals[:, c0:c0 + TOPK]
                        src = pt
                        if _VARIANT == "sbuf":
                            st = stage.tile([PART, GRP], f32)
                            nc.scalar.copy(st[:], pt[:])
                            src = st
                        if _ABLATE >= 2:
                            # keep the matmuls live with a minimal psum read
                            nc.vector.tensor_copy(vals[:, c0:c0 + 1], pt[:, 0:1])
                        if _ABLATE < 2:
                            nc.vector.max(out=vs, in_=src[:])
                        if _ABLATE < 1:
                            nc.vector.max_index(
                                out=idxs[:, c0:c0 + TOPK], in_max=vs, in_values=src[:]
                            )
                    if _VARIANT == "strip":
                        c0 = m * TOPK
                        vs = vals[:, c0:c0 + TOPK]
                        nc.vector.max(out=vs, in_=strip[:])
                        nc.vector.max_index(
                            out=idxs[:, c0:c0 + TOPK], in_max=vs, in_values=strip[:]
                        )
                    if _VARIANT == "sbuf2":
                        for g in range(NG):
                            c0 = m * CAND + g * TOPK
                            nc.vector.max(out=vals[:, c0:c0 + TOPK], in_=sts[g][:])
                        for g in range(NG):
                            c0 = m * CAND + g * TOPK
                            nc.vector.max_index(
                                out=idxs[:, c0:c0 + TOPK],
                                in_max=vals[:, c0:c0 + TOPK],
                                in_values=sts[g][:],
                            )

            if loops > 1:
                with tc.For_i(0, loops, 1):
                    body()
            else:
                body()

            nc.sync.dma_start(vals_d.ap(), vals[:])
            nc.scalar.dma_start(idxs_d.ap(), idxs[:])

    nc.compile()
    return nc


def _get_program():
    if "nc" not in _prog_cache:
        _prog_cache["nc"] = _build_program()
    return _prog_cache["nc"]


def _dist32(sim):
    """Reference fp32 distance pipeline: sqrt2 * sqrt(clip(1 - sim, 1e-6))."""
    sim = np.asarray(sim, dtype=np.float32)
    t = np.clip(np.float32(1.0) - sim, np.float32(1e-6), None).astype(np.float32)
    return (SQRT_2 * np.sqrt(t)).astype(np.float32)


def _select_winners(vals, idxs, a64, b64):
    """Pick per-row argmin-of-dist winners from top-8-per-group candidates.

    vals, idxs: [PART, NSTRIP*CAND] device outputs for one core.
    a64, b64: fp64 copies of the descriptor sets (rows of S are a64 @ b64.T),
      used to refine rows where fp32 candidate sims are too close to call.
    Returns (win_idx int64 [M], win_sim float32 [M]).
    """
    # [p, m, g, k] -> row r = m*PART + p; group offsets per variant
    ng, gw = _GROUPS
    V = vals.reshape(PART, NSTRIP, ng, TOPK).transpose(1, 0, 2, 3).reshape(M, CAND)
    I = idxs.reshape(PART, NSTRIP, ng, TOPK).transpose(1, 0, 2, 3).astype(np.int64)
    I += np.arange(ng, dtype=np.int64)[None, None, :, None] * gw
    I = I.reshape(M, CAND)

    rows = np.arange(M)

    def pick(Vc, Ic):
        dist = _dist32(Vc)
        dmin = dist.min(axis=1, keepdims=True)
        tie = dist == dmin
        gi = np.where(tie, Ic, np.int64(1) << 40)
        widx = gi.min(axis=1)
        wpos = np.argmax(tie & (gi == widx[:, None]), axis=1)
        return widx, Vc[rows[: len(Vc)], wpos]

    win_idx, win_sim = pick(V, I)

    # Rows where several candidates sit within fp32-rounding distance of the
    # max: recompute their candidate sims in fp64 and redo the fp32 pipeline,
    # mirroring what the reference's own fp32 matmul would produce.
    vmax = V.max(axis=1, keepdims=True)
    near = (vmax - V) < np.float32(3e-5)
    amb = np.flatnonzero(near.sum(axis=1) > 1)
    if os.environ.get("KERNEL_DEBUG"):
        print(f"[kernel] rows fp64-refined: {amb.size}/{len(V)}")
    if amb.size:
        Ic = np.clip(I[amb], 0, b64.shape[0] - 1)
        sims64 = np.einsum(
            "rd,rcd->rc", a64[amb], b64[Ic], optimize=True
        )
        V2 = sims64.astype(np.float32)
        w2, s2 = pick(V2, I[amb])
        win_idx[amb] = w2
        win_sim[amb] = s2

    return win_idx, win_sim


def _match_batch_host(row_res, col_res, d0b, d1b):
    """Reproduce reference _match_batch from the two cores' candidate lists."""
    d0_64 = d0b.astype(np.float64)
    d1_64 = d1b.astype(np.float64)
    n_amin, sim_row = _select_winners(row_res["vals"], row_res["idxs"], d0_64, d1_64)
    m_amin, _ = _select_winners(col_res["vals"], col_res["idxs"], d1_64, d0_64)

    rng_m = np.arange(M, dtype=np.int64)
    mask = m_amin[n_amin] == rng_m

    dist_w = _dist32(sim_row)
    score = (np.float32(1.0) / (np.float32(1.0) + dist_w)).astype(np.float32)

    m0 = np.where(mask, n_amin, -1).astype(np.int32)
    ms0 = np.where(mask, score, np.float32(0.0)).astype(np.float32)

    m1 = np.full(N, -1, dtype=np.int32)
    ms1 = np.zeros(N, dtype=np.float32)
    sel = np.flatnonzero(mask)
    m1[n_amin[sel]] = sel.astype(np.int32)
    ms1[n_amin[sel]] = score[sel]
    return m0, ms0, m1, ms1


def _build_in_maps(desc0, desc1):
    d0T = np.ascontiguousarray(desc0.transpose(0, 2, 1))  # [B, 64, M]
    d1T = np.ascontiguousarray(desc1.transpose(0, 2, 1))  # [B, 64, N]
    in_maps = []
    for b in range(B):
        in_maps.append({"at": d0T[b], "bt": d1T[b]})  # row side (o=0)
        in_maps.append({"at": d1T[b], "bt": d0T[b]})  # col side (o=1)
    return in_maps


def run_device(in_maps, trace=False):
    nc = _get_program()
    return run_bass_kernel_spmd(nc, in_maps, core_ids=list(range(8)), trace=trace)


def kernel(kpts0, desc0, kpts1, desc1):
    desc0 = np.asarray(desc0, dtype=np.float32)
    desc1 = np.asarray(desc1, dtype=np.float32)
    assert desc0.shape == (B, M, D) and desc1.shape == (B, N, D)

    in_maps = _build_in_maps(desc0, desc1)
    trace = bool(int(os.environ.get("KERNEL_PROFILE", "0")))
    res = run_device(in_maps, trace=trace)
    kernel.last_results = res
    kernel.last_exec_time_ns = res.exec_time_ns

    m0 = np.empty((B, M), np.int32)
    ms0 = np.empty((B, M), np.float32)
    m1 = np.empty((B, N), np.int32)
    ms1 = np.empty((B, N), np.float32)
    for b in range(B):
        m0[b], ms0[b], m1[b], ms1[b] = _match_batch_host(
            res.results[2 * b], res.results[2 * b + 1], desc0[b], desc1[b]
        )
    return m0, ms0, m1, ms1



# revision 4
# speedup vs baseline: 4.2199x; 4.2199x over previous
"""CycleMatcher (mutual-nearest-neighbor descriptor matching) on trn2.

Problem: B=4 pairs of L2-normalized descriptor sets d0,d1 [8192, 64].
dist = sqrt2*sqrt(clip(1 - d0@d1.T, 1e-6)); row/col argmins; mutual-NN
masking; scatter. dist is monotone-decreasing in sim = d0@d1.T, so argmin
dist == argmax sim (fp32 sqrt-rounding ties resolved on host in fp64).

The device kernel (~2.4 ms) is dwarfed by the axon PJRT tunnel cost
(~80 ms/dispatch fixed + ~80 MB/s each way), so the design minimizes
bytes moved and dispatches:

- Sharding: 4 cores, one batch each; every core computes BOTH match
  directions (S = d0@d1.T row-argmax candidates and S.T row-argmax
  candidates), so each batch's descriptors are uploaded exactly once.
- Inputs are sent as ONE fp16 tensor per core [128, 8192]: partitions
  0-63 hold d0[b].T, 64-127 hold d1[b].T (8 MB total up vs 32 MB for the
  fp32 8-core layout). fp16 input rounding perturbs sims by ~1e-4 which
  the host-side fp64 refinement absorbs (see REFINE_TH).
- Outputs are ONE u16 tensor per core [128, 8192]: columns 0:4096 =
  group-local argmax indices (u16 straight from DVE max_index), columns
  4096:8192 = top-8 group sims cast to fp16 and bitcast to u16
  (8 MB total down vs 16 MB).
- Device program per direction: 64 row-strips x [64,128]^T @ [64,512]
  fp16 matmuls fill [128,2048] PSUM groups (double buffered); ScalarE
  drains each group to SBUF; DVE max (top-8) + max_index per group.
- Dispatch: a module-cached jax.jit(shard_map(bass_exec)) instead of
  run_bass_kernel_spmd, which rebuilds + retraces the jit every call
  (~300 ms). Donated output zero-buffers are created ON DEVICE by a
  second cached jit, so no 16 MB zero upload per call.

Host merges each row's 32 candidates (4 groups x top-8), resolves
fp16/fp32 rounding near-ties exactly in reference fp32 semantics via
fp64 recompute, then does the cheap mutual-NN match + scatter in numpy.
"""

import os
import sys

# Prefer whatever copy PYTHONPATH already provides (the axon sitecustomize
# puts /root/.axon_site/_ro/trn_rl_repo there); append fallbacks so kernel.py
# also works standalone without creating dual module identities.
for _p in ("/root/.axon_site/_ro/trn_rl_repo", "/opt/trn_rl_repo"):
    if _p not in sys.path:
        sys.path.append(_p)

import numpy as np

import concourse.bass as bass  # noqa: F401  (registers lowerings)
import concourse.mybir as mybir
import concourse.tile as tile
from concourse import bacc

B = 4
M = 8192
N = 8192
D = 64

NCORES = 4
PART = 128          # rows per strip (psum partitions)
NSTRIP = M // PART  # 64
MMN = 512           # matmul moving free dim (one psum bank, fp32 psum)
GRP = 2048          # psum group / DVE op width
NG = N // GRP       # 4 groups per strip
TOPK = 8            # DVE max/max_index width
CAND = NG * TOPK    # 32 candidates per row
HALF = NSTRIP * CAND  # 2048 output columns per direction

SQRT_2 = np.float32(1.414213)

# Host fp64-refine threshold on candidate sims. Must cover 2x the worst
# device-vs-exact sim deviation: fp16 input rounding (~1e-4 rms, <5e-4
# tail) + fp16 export rounding (<= 4.9e-4 at |sim|<1). 4e-3 gives >4x
# margin; ~10-25% of rows refine, a cheap numpy einsum.
REFINE_TH = np.float32(float(os.environ.get("KERNEL_REFINE_TH", "4e-3")))

_cache = {}


def _build_program():
    nc = bacc.Bacc("TRN2", target_bir_lowering=False, debug=False)
    f16 = mybir.dt.float16
    f32 = mybir.dt.float32
    u16 = mybir.dt.uint16

    ab_d = nc.dram_tensor("ab", [2 * D, M], f16, kind="ExternalInput")
    out_d = nc.dram_tensor("out", [PART, 2 * 2 * HALF], u16, kind="ExternalOutput")

    with tile.TileContext(nc) as tc:
        with (
            tc.tile_pool(name="inp", bufs=1) as inp,
            tc.tile_pool(name="outp", bufs=1) as outp,
            tc.tile_pool(name="ps", bufs=2, space="PSUM") as ps,
            tc.tile_pool(name="stage", bufs=4) as stage,
        ):
            # Two [64, M] tiles (both base partition 0 — the PE requires
            # matmul operands to share their base partition) filled from
            # the halves of the single concatenated input tensor.
            a0 = inp.tile([D, M], f16)
            b0 = inp.tile([D, M], f16)
            nc.sync.dma_start(a0[:], ab_d.ap()[0:D, :])
            nc.scalar.dma_start(b0[:], ab_d.ap()[D:2 * D, :])

            out_t = outp.tile([PART, 2 * 2 * HALF], u16)   # [128, 8192]
            vals32 = outp.tile([PART, 2 * HALF], f32)      # [128, 4096]

            for d in range(2):
                at = a0 if d == 0 else b0
                bt = b0 if d == 0 else a0
                for m in range(NSTRIP):
                    lhsT = at[:, m * PART:(m + 1) * PART]  # [64, 128] stationary
                    for g in range(NG):
                        pt = ps.tile([PART, GRP], f32)
                        for j in range(GRP // MMN):
                            n0 = g * GRP + j * MMN
                            nc.tensor.matmul(
                                pt[:, j * MMN:(j + 1) * MMN],
                                lhsT,
                                bt[:, n0:n0 + MMN],
                                start=True,
                                stop=True,
                            )
                        # ScalarE drain: DVE ops from PSUM pay extra access
                        # overhead and contend with PE writes.
                        st = stage.tile([PART, GRP], f32)
                        nc.scalar.copy(st[:], pt[:])
                        c0 = ((d * NSTRIP + m) * NG + g) * TOPK
                        vs = vals32[:, c0:c0 + TOPK]
                        nc.vector.max(out=vs, in_=st[:])
                        nc.vector.max_index(
                            out=out_t[:, c0:c0 + TOPK], in_max=vs, in_values=st[:]
                        )

            # Export sims as fp16 into the u16 output tensor's upper half.
            vals16 = out_t[:, 2 * HALF:4 * HALF].bitcast(mybir.dt.float16)
            nc.scalar.copy(vals16, vals32[:])
            nc.sync.dma_start(out_d.ap(), out_t[:])

    nc.compile()
    return nc


def _get_dispatcher():
    """Build (once) the jitted shard_map dispatch + device-side zeros maker.

    Replicates concourse.bass2jax.run_bass_via_pjrt but caches the jitted
    callable (run_bass_via_pjrt re-creates and re-traces it every call) and
    sources the donated output buffers from an on-device jnp.zeros jit
    instead of uploading host zeros through the tunnel.
    """
    if "disp" in _cache:
        return _cache["disp"]

    import jax
    import jax.numpy as jnp
    from jax.experimental.shard_map import shard_map
    from jax.sharding import Mesh, NamedSharding, PartitionSpec

    from concourse.bass2jax import (
        _bass_exec_p,
        install_neuronx_cc_hook,
        partition_id_tensor,
    )

    nc = _build_program()
    install_neuronx_cc_hook()

    partition_name = nc.partition_id_tensor.name if nc.partition_id_tensor else None
    in_names = []
    out_names = []
    out_avals = []
    out_np = []
    for alloc in nc.m.functions[0].allocations:
        if not isinstance(alloc, mybir.MemoryLocationSet):
            continue
        name = alloc.memorylocations[0].name
        if alloc.kind == "ExternalInput":
            if name != partition_name:
                in_names.append(name)
        elif alloc.kind == "ExternalOutput":
            shape = tuple(alloc.tensor_shape)
            dtype = mybir.dt.np(alloc.dtype)
            out_names.append(name)
            out_avals.append(jax.core.ShapedArray(shape, dtype))
            out_np.append((shape, dtype))
    n_params = len(in_names)
    all_names = tuple(in_names) + tuple(out_names)
    if partition_name is not None:
        all_names = all_names + (partition_name,)

    def _body(*args):
        operands = list(args)
        if partition_name is not None:
            operands.append(partition_id_tensor())
        outs = _bass_exec_p.bind(
            *operands,
            out_avals=tuple(out_avals),
            in_names=all_names,
            out_names=tuple(out_names),
            lowering_input_output_aliases=(),
            sim_require_finite=True,
            sim_require_nnan=True,
            nc=nc,
        )
        return tuple(outs)

    devices = jax.devices()[:NCORES]
    mesh = Mesh(np.asarray(devices), ("core",))
    spec = PartitionSpec("core")
    nin = n_params + len(out_names)
    sharded = jax.jit(
        shard_map(
            _body,
            mesh=mesh,
            in_specs=(spec,) * nin,
            out_specs=(spec,) * len(out_names),
            check_rep=False,
        ),
        donate_argnums=tuple(range(n_params, nin)),
        keep_unused=True,
    )

    shz = NamedSharding(mesh, spec)
    zero_shapes = [((NCORES * s[0],) + s[1:], d) for s, d in out_np]
    make_zeros = jax.jit(
        lambda: tuple(jnp.zeros(s, d) for s, d in zero_shapes),
        out_shardings=(shz,) * len(zero_shapes),
    )

    _cache["disp"] = (sharded, make_zeros)
    return _cache["disp"]


def prep_inputs(desc0, desc1):
    """fp32 [B, M, D] descriptor pair -> concat fp16 device input [B*128, M]."""
    ab = np.empty((B, 2 * D, M), np.float16)
    ab[:, :D] = desc0.transpose(0, 2, 1)
    ab[:, D:] = desc1.transpose(0, 2, 1)
    return ab.reshape(B * 2 * D, M)


def run_device(ab_all):
    """One device dispatch: [512, 8192] f16 in -> [512, 8192] u16 out (np)."""
    sharded, make_zeros = _get_dispatcher()
    zeros = make_zeros()
    (out,) = sharded(ab_all, *zeros)
    return np.asarray(out).reshape(NCORES, PART, 2 * 2 * HALF)


def _dist32(sim):
    """Reference fp32 distance pipeline: sqrt2 * sqrt(clip(1 - sim, 1e-6))."""
    sim = np.asarray(sim, dtype=np.float32)
    t = np.clip(np.float32(1.0) - sim, np.float32(1e-6), None).astype(np.float32)
    return (SQRT_2 * np.sqrt(t)).astype(np.float32)


def _select_winners(vals, idxs, a64, b64):
    """Pick per-row argmin-of-dist winners from top-8-per-group candidates.

    vals: [PART, HALF] fp16 sims, idxs: [PART, HALF] u16 group-local indices
    for one core+direction. a64, b64: fp64 descriptor sets (rows of S are
    a64 @ b64.T), used to (a) refine rows whose fp16 candidate sims are too
    close to call in exact reference fp32 semantics, (b) recompute every
    winner's sim at full precision for the score.
    Returns (win_idx int64 [M], win_sim float32 [M]).
    """
    # [p, m, g, k] -> row r = m*PART + p, global col = g*GRP + local
    V = (
        vals.reshape(PART, NSTRIP, NG, TOPK)
        .transpose(1, 0, 2, 3)
        .reshape(M, CAND)
        .astype(np.float32)
    )
    I = idxs.reshape(PART, NSTRIP, NG, TOPK).transpose(1, 0, 2, 3).astype(np.int64)
    I += np.arange(NG, dtype=np.int64)[None, None, :, None] * GRP
    I = I.reshape(M, CAND)

    rows = np.arange(M)

    def pick(Vc, Ic):
        dist = _dist32(Vc)
        dmin = dist.min(axis=1, keepdims=True)
        tie = dist == dmin
        gi = np.where(tie, Ic, np.int64(1) << 40)
        widx = gi.min(axis=1)
        return widx

    win_idx = pick(V, I)

    # Rows where several candidates sit within fp16-rounding distance of the
    # max: recompute their candidate sims in fp64 and redo the fp32 pipeline,
    # mirroring what the reference's own fp32 matmul would produce.
    vmax = V.max(axis=1, keepdims=True)
    near = (vmax - V) < REFINE_TH
    amb = np.flatnonzero(near.sum(axis=1) > 1)
    if os.environ.get("KERNEL_DEBUG"):
        print(f"[kernel] rows fp64-refined: {amb.size}/{len(V)}")
    if amb.size:
        Ic = np.clip(I[amb], 0, b64.shape[0] - 1)
        sims64 = np.einsum("rd,rcd->rc", a64[amb], b64[Ic], optimize=True)
        win_idx[amb] = pick(sims64.astype(np.float32), I[amb])

    # Winner sims at full precision -> fp32 (reference-grade accuracy).
    win_sim = (
        np.einsum("rd,rd->r", a64, b64[win_idx])
        .astype(np.float32)
    )
    return win_idx, win_sim


def _match_batch_host(core_out, d0b, d1b):
    """Reproduce reference _match_batch from one core's candidate tensor."""
    idx_u16 = core_out[:, : 2 * HALF]
    val_f16 = core_out[:, 2 * HALF:].view(np.float16)
    d0_64 = d0b.astype(np.float64)
    d1_64 = d1b.astype(np.float64)
    n_amin, sim_row = _select_winners(
        val_f16[:, :HALF], idx_u16[:, :HALF], d0_64, d1_64
    )
    m_amin, _ = _select_winners(
        val_f16[:, HALF:], idx_u16[:, HALF:], d1_64, d0_64
    )

    rng_m = np.arange(M, dtype=np.int64)
    mask = m_amin[n_amin] == rng_m

    dist_w = _dist32(sim_row)
    score = (np.float32(1.0) / (np.float32(1.0) + dist_w)).astype(np.float32)

    m0 = np.where(mask, n_amin, -1).astype(np.int32)
    ms0 = np.where(mask, score, np.float32(0.0)).astype(np.float32)

    m1 = np.full(N, -1, dtype=np.int32)
    ms1 = np.zeros(N, dtype=np.float32)
    sel = np.flatnonzero(mask)
    m1[n_amin[sel]] = sel.astype(np.int32)
    ms1[n_amin[sel]] = score[sel]
    return m0, ms0, m1, ms1


def kernel(kpts0, desc0, kpts1, desc1):
    desc0 = np.asarray(desc0, dtype=np.float32)
    desc1 = np.asarray(desc1, dtype=np.float32)
    assert desc0.shape == (B, M, D) and desc1.shape == (B, N, D)

    ab_all = prep_inputs(desc0, desc1)
    out = run_device(ab_all)
    kernel.last_results = out
    kernel.last_exec_time_ns = None

    m0 = np.empty((B, M), np.int32)
    ms0 = np.empty((B, M), np.float32)
    m1 = np.empty((B, N), np.int32)
    ms1 = np.empty((B, N), np.float32)
    for b in range(B):
        m0[b], ms0[b], m1[b], ms1[b] = _match_batch_host(
            out[b], desc0[b], desc1[b]
        )
    return m0, ms0, m1, ms1


# revision 7
# speedup vs baseline: 6.8209x; 1.6163x over previous
"""CycleMatcher (mutual-nearest-neighbor descriptor matching) on trn2.

Problem: B=4 pairs of L2-normalized descriptor sets d0,d1 [8192, 64].
dist = sqrt2*sqrt(clip(1 - d0@d1.T, 1e-6)); row/col argmins; mutual-NN
masking; scatter. dist is monotone-decreasing in sim = d0@d1.T, so argmin
dist == argmax sim (fp32 sqrt-rounding ties resolved on host in fp64).

The device kernel (~ a few ms) is dwarfed by the axon PJRT tunnel cost
(~60 ms/dispatch fixed + ~70 MB/s each way), so the design minimizes
bytes moved and dispatches:

- Sharding: 4 cores, one batch each; every core computes BOTH match
  directions (S = d0@d1.T row-argmax candidates and S.T row-argmax
  candidates), so each batch's descriptors are uploaded exactly once.
- Inputs are sent as ONE fp16 tensor per core [128, 8192]: partitions
  0-63 hold d0[b].T, 64-127 hold d1[b].T (8 MB total up vs 32 MB for the
  fp32 8-core layout). fp16 input rounding perturbs sims by ~1e-4, which
  the host-side fp64 refinement absorbs (see REFINE_TH).
- Outputs are 8 packed u32 PER ROW (2 MB total down vs 16 MB): the
  ScalarE PSUM drain computes sim+1.0 (maps sims into [1,2) where the
  IEEE fp32 bit pattern is monotone), DVE masks the low 13 mantissa bits
  and ORs in the column index (an iota), and a row-wide DVE max8 then
  yields the top-8 (quantized-sim, index) candidates in one value each.
  All 8192 packed row values are distinct (index bits), so max8 returns
  8 distinct columns, compared as positive fp32.
- Device program per direction: 64 row-strips x [64,128]^T @ [64,512]
  fp16 matmuls fill [128,2048] PSUM groups (double buffered); ScalarE
  drains+biases each group to SBUF; DVE packs and reduces.
- Dispatch: a module-cached jax.jit(shard_map(bass_exec)) instead of
  run_bass_kernel_spmd, which rebuilds + retraces the jit every call
  (~300 ms). Donated output zero-buffers are created ON DEVICE by a
  second cached jit, so no zero upload per call.

Host merges each row's 8 candidates, resolves fp16/quantization
near-ties exactly in reference fp32 semantics via fp64 recompute of the
candidate sims, recomputes every winner's sim in fp64->fp32 for the
score, then does the cheap mutual-NN match + scatter in numpy.
"""

import os
import sys

# Prefer whatever copy PYTHONPATH already provides (the axon sitecustomize
# puts /root/.axon_site/_ro/trn_rl_repo there); append fallbacks so kernel.py
# also works standalone without creating dual module identities.
for _p in ("/root/.axon_site/_ro/trn_rl_repo", "/opt/trn_rl_repo"):
    if _p not in sys.path:
        sys.path.append(_p)

import numpy as np

import concourse.bass as bass  # noqa: F401  (registers lowerings)
import concourse.mybir as mybir
import concourse.tile as tile
from concourse import bacc

B = 4
M = 8192
N = 8192
D = 64

NCORES = 4
PART = 128          # rows per strip (psum partitions)
NSTRIP = M // PART  # 64
MMN = 512           # matmul moving free dim (one psum bank, fp32 psum)
GRP = 2048          # psum group width
NG = N // GRP       # 4 groups per strip
TOPK = 8            # DVE max8 width = candidates per row
OUTW = 2 * NSTRIP * TOPK  # 1024 output cols per core (2 directions)

IDX_BITS = 13
IDX_MASK = (1 << IDX_BITS) - 1          # 0x1FFF
QUANT_MASK = 0xFFFFFFFF ^ IDX_MASK      # keep sign+exp+10 mantissa bits

SQRT_2 = np.float32(1.414213)

# Host fp64-refine threshold on candidate sims. Must cover 2x the worst
# device-vs-exact sim deviation: fp16 input rounding (~1e-4 rms, <5e-4
# tail) + 13-bit packing quantization (<= 2^-10 ~ 9.8e-4, downward).
# 4e-3 gives >2.5x margin; ~10-25% of rows refine, a cheap numpy einsum.
REFINE_TH = np.float32(float(os.environ.get("KERNEL_REFINE_TH", "4e-3")))

_cache = {}


def _build_program():
    nc = bacc.Bacc("TRN2", target_bir_lowering=False, debug=False)
    f16 = mybir.dt.float16
    f32 = mybir.dt.float32
    u32 = mybir.dt.uint32

    ab_d = nc.dram_tensor("ab", [2 * D, M], f16, kind="ExternalInput")
    out_d = nc.dram_tensor("out", [PART, OUTW], f32, kind="ExternalOutput")

    with tile.TileContext(nc) as tc:
        with (
            tc.tile_pool(name="inp", bufs=1) as inp,
            tc.tile_pool(name="outp", bufs=1) as outp,
            tc.tile_pool(name="ps", bufs=2, space="PSUM") as ps,
            tc.tile_pool(name="stage", bufs=4) as stage,
            tc.tile_pool(name="strip", bufs=2) as strippool,
        ):
            # Two [64, M] tiles (both base partition 0 — the PE requires
            # matmul operands to share their base partition) filled from
            # the halves of the single concatenated input tensor.
            a0 = inp.tile([D, M], f16)
            b0 = inp.tile([D, M], f16)
            nc.sync.dma_start(a0[:], ab_d.ap()[0:D, :])
            nc.scalar.dma_start(b0[:], ab_d.ap()[D:2 * D, :])

            # Global column index, identical on every partition.
            it = inp.tile([PART, M], u32)
            nc.gpsimd.iota(it[:], [[1, M]], channel_multiplier=0)

            top8 = outp.tile([PART, OUTW], f32)

            for d in range(2):
                at = a0 if d == 0 else b0
                bt = b0 if d == 0 else a0
                for m in range(NSTRIP):
                    lhsT = at[:, m * PART:(m + 1) * PART]  # [64, 128] stationary
                    pk = strippool.tile([PART, M], u32)
                    pkf = pk[:].bitcast(f32)
                    for g in range(NG):
                        pt = ps.tile([PART, GRP], f32)
                        for j in range(GRP // MMN):
                            n0 = g * GRP + j * MMN
                            nc.tensor.matmul(
                                pt[:, j * MMN:(j + 1) * MMN],
                                lhsT,
                                bt[:, n0:n0 + MMN],
                                start=True,
                                stop=True,
                            )
                        # ScalarE drain with +1.0 bias: sims -> [1, 2) where
                        # the fp32 bit pattern is monotone in the value.
                        st = stage.tile([PART, GRP], f32)
                        nc.scalar.add(st[:], pt[:], 1.0)
                        gsl = pk[:, g * GRP:(g + 1) * GRP]
                        # quantize (drop low 13 mantissa bits) ...
                        nc.vector.tensor_scalar(
                            gsl,
                            st[:].bitcast(u32),
                            QUANT_MASK,
                            None,
                            mybir.AluOpType.bitwise_and,
                        )
                        # ... and OR in the global column index.
                        nc.vector.tensor_tensor(
                            gsl,
                            gsl,
                            it[:, g * GRP:(g + 1) * GRP],
                            mybir.AluOpType.bitwise_or,
                        )
                    c0 = (d * NSTRIP + m) * TOPK
                    nc.vector.max(out=top8[:, c0:c0 + TOPK], in_=pkf)

            nc.sync.dma_start(out_d.ap(), top8[:])

    nc.compile()
    return nc


def _get_dispatcher():
    """Build (once) the jitted shard_map dispatch + device-side zeros maker.

    Replicates concourse.bass2jax.run_bass_via_pjrt but caches the jitted
    callable (run_bass_via_pjrt re-creates and re-traces it every call) and
    sources the donated output buffers from an on-device jnp.zeros jit
    instead of uploading host zeros through the tunnel.
    """
    if "disp" in _cache:
        return _cache["disp"]

    import jax
    import jax.numpy as jnp
    from jax.experimental.shard_map import shard_map
    from jax.sharding import Mesh, NamedSharding, PartitionSpec

    from concourse.bass2jax import (
        _bass_exec_p,
        install_neuronx_cc_hook,
        partition_id_tensor,
    )

    nc = _build_program()
    install_neuronx_cc_hook()

    partition_name = nc.partition_id_tensor.name if nc.partition_id_tensor else None
    in_names = []
    out_names = []
    out_avals = []
    out_np = []
    for alloc in nc.m.functions[0].allocations:
        if not isinstance(alloc, mybir.MemoryLocationSet):
            continue
        name = alloc.memorylocations[0].name
        if alloc.kind == "ExternalInput":
            if name != partition_name:
                in_names.append(name)
        elif alloc.kind == "ExternalOutput":
            shape = tuple(alloc.tensor_shape)
            dtype = mybir.dt.np(alloc.dtype)
            out_names.append(name)
            out_avals.append(jax.core.ShapedArray(shape, dtype))
            out_np.append((shape, dtype))
    n_params = len(in_names)
    all_names = tuple(in_names) + tuple(out_names)
    if partition_name is not None:
        all_names = all_names + (partition_name,)

    def _body(*args):
        operands = list(args)
        if partition_name is not None:
            operands.append(partition_id_tensor())
        outs = _bass_exec_p.bind(
            *operands,
            out_avals=tuple(out_avals),
            in_names=all_names,
            out_names=tuple(out_names),
            lowering_input_output_aliases=(),
            sim_require_finite=True,
            sim_require_nnan=True,
            nc=nc,
        )
        return tuple(outs)

    devices = jax.devices()[:NCORES]
    mesh = Mesh(np.asarray(devices), ("core",))
    spec = PartitionSpec("core")
    nin = n_params + len(out_names)
    sharded = jax.jit(
        shard_map(
            _body,
            mesh=mesh,
            in_specs=(spec,) * nin,
            out_specs=(spec,) * len(out_names),
            check_rep=False,
        ),
        donate_argnums=tuple(range(n_params, nin)),
        keep_unused=True,
    )

    shz = NamedSharding(mesh, spec)
    zero_shapes = [((NCORES * s[0],) + s[1:], d) for s, d in out_np]
    make_zeros = jax.jit(
        lambda: tuple(jnp.zeros(s, d) for s, d in zero_shapes),
        out_shardings=(shz,) * len(zero_shapes),
    )

    _cache["disp"] = (sharded, make_zeros)
    return _cache["disp"]


def prep_inputs(desc0, desc1):
    """fp32 [B, M, D] descriptor pair -> concat fp16 device input [B*128, M]."""
    ab = np.empty((B, 2 * D, M), np.float16)
    ab[:, :D] = desc0.transpose(0, 2, 1)
    ab[:, D:] = desc1.transpose(0, 2, 1)
    return ab.reshape(B * 2 * D, M)


def run_device(ab_all):
    """One device dispatch: [512, 8192] f16 in -> [4, 128, 1024] u32 out."""
    sharded, make_zeros = _get_dispatcher()
    zeros = make_zeros()
    (out,) = sharded(ab_all, *zeros)
    return np.asarray(out).view(np.uint32).reshape(NCORES, PART, OUTW)


def _dist32(sim):
    """Reference fp32 distance pipeline: sqrt2 * sqrt(clip(1 - sim, 1e-6))."""
    sim = np.asarray(sim, dtype=np.float32)
    t = np.clip(np.float32(1.0) - sim, np.float32(1e-6), None).astype(np.float32)
    return (SQRT_2 * np.sqrt(t)).astype(np.float32)


def _select_winners(packed, a64, b64):
    """Pick per-row argmin-of-dist winners from top-8 packed candidates.

    packed: [PART, NSTRIP*TOPK] u32 for one core+direction; each value is
    (bits(sim+1) & QUANT_MASK) | column. a64, b64: fp64 descriptor sets
    (rows of S are a64 @ b64.T), used to (a) refine rows whose quantized
    candidate sims are too close to call, in exact reference fp32
    semantics, (b) recompute every winner's sim for the score.
    Returns (win_idx int64 [M], win_sim float32 [M]).
    """
    # [p, m, k] -> row r = m*PART + p
    T = packed.reshape(PART, NSTRIP, TOPK).transpose(1, 0, 2).reshape(M, TOPK)
    I = (T & np.uint32(IDX_MASK)).astype(np.int64)
    V = (T & np.uint32(QUANT_MASK)).view(np.float32) - np.float32(1.0)

    def pick(Vc, Ic):
        dist = _dist32(Vc)
        dmin = dist.min(axis=1, keepdims=True)
        tie = dist == dmin
        gi = np.where(tie, Ic, np.int64(1) << 40)
        return gi.min(axis=1)

    win_idx = pick(V, I)

    # Rows where several candidates sit within quantization distance of the
    # max: recompute their candidate sims in fp64 and redo the fp32 pipeline,
    # mirroring what the reference's own fp32 matmul would produce.
    vmax = V.max(axis=1, keepdims=True)
    near = (vmax - V) < REFINE_TH
    amb = np.flatnonzero(near.sum(axis=1) > 1)
    if os.environ.get("KERNEL_DEBUG"):
        print(f"[kernel] rows fp64-refined: {amb.size}/{len(V)}")
    if amb.size:
        sims64 = np.einsum("rd,rcd->rc", a64[amb], b64[I[amb]], optimize=True)
        win_idx[amb] = pick(sims64.astype(np.float32), I[amb])

    # Winner sims at full precision -> fp32 (reference-grade accuracy).
    win_sim = np.einsum("rd,rd->r", a64, b64[win_idx]).astype(np.float32)
    return win_idx, win_sim


def _match_batch_host(core_out, d0b, d1b):
    """Reproduce reference _match_batch from one core's candidate tensor."""
    d0_64 = d0b.astype(np.float64)
    d1_64 = d1b.astype(np.float64)
    half = NSTRIP * TOPK
    n_amin, sim_row = _select_winners(core_out[:, :half], d0_64, d1_64)
    m_amin, _ = _select_winners(core_out[:, half:], d1_64, d0_64)

    rng_m = np.arange(M, dtype=np.int64)
    mask = m_amin[n_amin] == rng_m

    dist_w = _dist32(sim_row)
    score = (np.float32(1.0) / (np.float32(1.0) + dist_w)).astype(np.float32)

    m0 = np.where(mask, n_amin, -1).astype(np.int32)
    ms0 = np.where(mask, score, np.float32(0.0)).astype(np.float32)

    m1 = np.full(N, -1, dtype=np.int32)
    ms1 = np.zeros(N, dtype=np.float32)
    sel = np.flatnonzero(mask)
    m1[n_amin[sel]] = sel.astype(np.int32)
    ms1[n_amin[sel]] = score[sel]
    return m0, ms0, m1, ms1


def kernel(kpts0, desc0, kpts1, desc1):
    desc0 = np.asarray(desc0, dtype=np.float32)
    desc1 = np.asarray(desc1, dtype=np.float32)
    assert desc0.shape == (B, M, D) and desc1.shape == (B, N, D)

    ab_all = prep_inputs(desc0, desc1)
    out = run_device(ab_all)
    kernel.last_results = out
    kernel.last_exec_time_ns = None

    m0 = np.empty((B, M), np.int32)
    ms0 = np.empty((B, M), np.float32)
    m1 = np.empty((B, N), np.int32)
    ms1 = np.empty((B, N), np.float32)
    for b in range(B):
        m0[b], ms0[b], m1[b], ms1[b] = _match_batch_host(
            out[b], desc0[b], desc1[b]
        )
    return m0, ms0, m1, ms1


# revision 9
# speedup vs baseline: 8.9926x; 1.3184x over previous
"""CycleMatcher (mutual-nearest-neighbor descriptor matching) on trn2.

Problem: B=4 pairs of L2-normalized descriptor sets d0,d1 [8192, 64].
dist = sqrt2*sqrt(clip(1 - d0@d1.T, 1e-6)); row/col argmins; mutual-NN
masking; scatter. dist is monotone-decreasing in sim = d0@d1.T, so argmin
dist == argmax sim (fp32 sqrt-rounding ties resolved on host in fp64).

The device kernel (~ a few ms) is dwarfed by the axon PJRT tunnel cost
(~60 ms/dispatch fixed + ~70 MB/s each way), so the design minimizes
bytes moved and dispatches:

- Sharding: 4 cores, one batch each; every core computes BOTH match
  directions (S = d0@d1.T row-argmax candidates and S.T row-argmax
  candidates), so each batch's descriptors are uploaded exactly once.
- Inputs are sent as ONE fp16 tensor per core [128, 8192]: partitions
  0-63 hold d0[b].T, 64-127 hold d1[b].T (8 MB total up vs 32 MB for the
  fp32 8-core layout). fp16 input rounding perturbs sims by ~1e-4, which
  the host-side fp64 refinement absorbs (see REFINE_TH).
- Outputs are 8 packed u32 PER ROW (2 MB total down vs 16 MB): the
  ScalarE PSUM drain computes sim+1.0 (maps sims into [1,2) where the
  IEEE fp32 bit pattern is monotone), DVE masks the low 13 mantissa bits
  and ORs in the column index (an iota), and a row-wide DVE max8 then
  yields the top-8 (quantized-sim, index) candidates in one value each.
  All 8192 packed row values are distinct (index bits), so max8 returns
  8 distinct columns, compared as positive fp32.
- Device program per direction: 64 row-strips x [64,128]^T @ [64,512]
  fp16 matmuls fill [128,2048] PSUM groups (double buffered); ScalarE
  drains+biases each group to SBUF; DVE packs and reduces.
- Dispatch: a module-cached jax.jit(shard_map(bass_exec)) instead of
  run_bass_kernel_spmd, which rebuilds + retraces the jit every call
  (~300 ms). Donated output zero-buffers are created ON DEVICE by a
  second cached jit, so no zero upload per call.

Host merges each row's 8 candidates, resolves fp16/quantization
near-ties exactly in reference fp32 semantics via fp64 recompute of the
candidate sims, recomputes every winner's sim in fp64->fp32 for the
score, then does the cheap mutual-NN match + scatter in numpy.
"""

import os
import sys

# Prefer whatever copy PYTHONPATH already provides (the axon sitecustomize
# puts /root/.axon_site/_ro/trn_rl_repo there); append fallbacks so kernel.py
# also works standalone without creating dual module identities.
for _p in ("/root/.axon_site/_ro/trn_rl_repo", "/opt/trn_rl_repo"):
    if _p not in sys.path:
        sys.path.append(_p)

import numpy as np

import concourse.bass as bass  # noqa: F401  (registers lowerings)
import concourse.mybir as mybir
import concourse.tile as tile
from concourse import bacc

B = 4
M = 8192
N = 8192
D = 64

NCORES = 4
PART = 128          # rows per strip (psum partitions)
NSTRIP = M // PART  # 64
MMN = 512           # matmul moving free dim (one psum bank, fp32 psum)
GRP = 2048          # psum group width
NG = N // GRP       # 4 groups per strip
TOPK = 8            # DVE max8 width = candidates per row
OUTW = 2 * NSTRIP * TOPK  # 1024 output cols per core (2 directions)

IDX_BITS = 13
IDX_MASK = (1 << IDX_BITS) - 1          # 0x1FFF
QUANT_MASK = 0xFFFFFFFF ^ IDX_MASK      # keep sign+exp+10 mantissa bits

SQRT_2 = np.float32(1.414213)

# Input wire format: "f16" (8 MB up) or "f8" (e4m3, 4 MB up). The PE
# accumulates either in fp32; coarser inputs only widen the band of rows
# the host must fp64-refine.
IN_DTYPE = os.environ.get("KERNEL_IN_DTYPE", "f16")

# Host fp64-refine threshold on candidate sims. Must cover 2x the worst
# device-vs-exact sim deviation: input rounding (f16: ~1e-4 rms, <5e-4
# tail; f8 e4m3: ~5.7e-3 rms, <2.5e-2 tail) + 13-bit packing quantization
# (<= 2^-10 ~ 9.8e-4, downward). f16: 4e-3 (~12% of rows refine);
# f8: 6e-2 (most rows refine — still a cheap numpy einsum).
REFINE_TH = np.float32(
    float(os.environ.get("KERNEL_REFINE_TH", "4e-3" if IN_DTYPE == "f16" else "6e-2"))
)

_cache = {}


def _build_program():
    nc = bacc.Bacc("TRN2", target_bir_lowering=False, debug=False)
    fin = mybir.dt.float16 if IN_DTYPE == "f16" else mybir.dt.float8e4
    f32 = mybir.dt.float32
    u32 = mybir.dt.uint32

    ab_d = nc.dram_tensor("ab", [2 * D, M], fin, kind="ExternalInput")
    out_d = nc.dram_tensor("out", [PART, OUTW], f32, kind="ExternalOutput")

    with tile.TileContext(nc) as tc:
        with (
            tc.tile_pool(name="inp", bufs=1) as inp,
            tc.tile_pool(name="outp", bufs=1) as outp,
            tc.tile_pool(name="ps", bufs=2, space="PSUM") as ps,
            tc.tile_pool(name="stage", bufs=4) as stage,
            tc.tile_pool(name="strip", bufs=2) as strippool,
        ):
            # Two [64, M] tiles (both base partition 0 — the PE requires
            # matmul operands to share their base partition) filled from
            # the halves of the single concatenated input tensor.
            a0 = inp.tile([D, M], fin)
            b0 = inp.tile([D, M], fin)
            nc.sync.dma_start(a0[:], ab_d.ap()[0:D, :])
            nc.scalar.dma_start(b0[:], ab_d.ap()[D:2 * D, :])

            # Global column index, identical on every partition.
            it = inp.tile([PART, M], u32)
            nc.gpsimd.iota(it[:], [[1, M]], channel_multiplier=0)

            top8 = outp.tile([PART, OUTW], f32)

            for d in range(2):
                at = a0 if d == 0 else b0
                bt = b0 if d == 0 else a0
                for m in range(NSTRIP):
                    lhsT = at[:, m * PART:(m + 1) * PART]  # [64, 128] stationary
                    pk = strippool.tile([PART, M], u32)
                    pkf = pk[:].bitcast(f32)
                    for g in range(NG):
                        pt = ps.tile([PART, GRP], f32)
                        for j in range(GRP // MMN):
                            n0 = g * GRP + j * MMN
                            nc.tensor.matmul(
                                pt[:, j * MMN:(j + 1) * MMN],
                                lhsT,
                                bt[:, n0:n0 + MMN],
                                start=True,
                                stop=True,
                            )
                        # ScalarE drain with +1.0 bias: sims -> [1, 2) where
                        # the fp32 bit pattern is monotone in the value.
                        st = stage.tile([PART, GRP], f32)
                        nc.scalar.add(st[:], pt[:], 1.0)
                        gsl = pk[:, g * GRP:(g + 1) * GRP]
                        # quantize (drop low 13 mantissa bits) ...
                        nc.vector.tensor_scalar(
                            gsl,
                            st[:].bitcast(u32),
                            QUANT_MASK,
                            None,
                            mybir.AluOpType.bitwise_and,
                        )
                        # ... and OR in the global column index.
                        nc.vector.tensor_tensor(
                            gsl,
                            gsl,
                            it[:, g * GRP:(g + 1) * GRP],
                            mybir.AluOpType.bitwise_or,
                        )
                    c0 = (d * NSTRIP + m) * TOPK
                    nc.vector.max(out=top8[:, c0:c0 + TOPK], in_=pkf)

            nc.sync.dma_start(out_d.ap(), top8[:])

    nc.compile()
    return nc


def _get_dispatcher():
    """Build (once) the jitted shard_map dispatch + device-side zeros maker.

    Replicates concourse.bass2jax.run_bass_via_pjrt but caches the jitted
    callable (run_bass_via_pjrt re-creates and re-traces it every call) and
    sources the donated output buffers from an on-device jnp.zeros jit
    instead of uploading host zeros through the tunnel.
    """
    if "disp" in _cache:
        return _cache["disp"]

    import jax
    import jax.numpy as jnp
    from jax.experimental.shard_map import shard_map
    from jax.sharding import Mesh, NamedSharding, PartitionSpec

    from concourse.bass2jax import (
        _bass_exec_p,
        install_neuronx_cc_hook,
        partition_id_tensor,
    )

    nc = _build_program()
    install_neuronx_cc_hook()

    partition_name = nc.partition_id_tensor.name if nc.partition_id_tensor else None
    in_names = []
    out_names = []
    out_avals = []
    out_np = []
    for alloc in nc.m.functions[0].allocations:
        if not isinstance(alloc, mybir.MemoryLocationSet):
            continue
        name = alloc.memorylocations[0].name
        if alloc.kind == "ExternalInput":
            if name != partition_name:
                in_names.append(name)
        elif alloc.kind == "ExternalOutput":
            shape = tuple(alloc.tensor_shape)
            dtype = mybir.dt.np(alloc.dtype)
            out_names.append(name)
            out_avals.append(jax.core.ShapedArray(shape, dtype))
            out_np.append((shape, dtype))
    n_params = len(in_names)
    all_names = tuple(in_names) + tuple(out_names)
    if partition_name is not None:
        all_names = all_names + (partition_name,)

    def _body(*args):
        operands = list(args)
        if partition_name is not None:
            operands.append(partition_id_tensor())
        outs = _bass_exec_p.bind(
            *operands,
            out_avals=tuple(out_avals),
            in_names=all_names,
            out_names=tuple(out_names),
            lowering_input_output_aliases=(),
            sim_require_finite=True,
            sim_require_nnan=True,
            nc=nc,
        )
        return tuple(outs)

    devices = jax.devices()[:NCORES]
    mesh = Mesh(np.asarray(devices), ("core",))
    spec = PartitionSpec("core")
    nin = n_params + len(out_names)
    sharded = jax.jit(
        shard_map(
            _body,
            mesh=mesh,
            in_specs=(spec,) * nin,
            out_specs=(spec,) * len(out_names),
            check_rep=False,
        ),
        donate_argnums=tuple(range(n_params, nin)),
        keep_unused=True,
    )

    shz = NamedSharding(mesh, spec)
    zero_shapes = [((NCORES * s[0],) + s[1:], d) for s, d in out_np]
    make_zeros = jax.jit(
        lambda: tuple(jnp.zeros(s, d) for s, d in zero_shapes),
        out_shardings=(shz,) * len(zero_shapes),
    )

    _cache["disp"] = (sharded, make_zeros)
    return _cache["disp"]


def prep_inputs(desc0, desc1):
    """fp32 [B, M, D] descriptor pair -> concat f16/f8 device input [B*128, M]."""
    np_in = mybir.dt.np(mybir.dt.float16 if IN_DTYPE == "f16" else mybir.dt.float8e4)
    ab = np.empty((B, 2 * D, M), np_in)
    ab[:, :D] = desc0.transpose(0, 2, 1)
    ab[:, D:] = desc1.transpose(0, 2, 1)
    return ab.reshape(B * 2 * D, M)


def run_device(ab_all):
    """One device dispatch: [512, 8192] f16 in -> [4, 128, 1024] u32 out."""
    sharded, make_zeros = _get_dispatcher()
    zeros = make_zeros()
    (out,) = sharded(ab_all, *zeros)
    return np.asarray(out).view(np.uint32).reshape(NCORES, PART, OUTW)


def _dist32(sim):
    """Reference fp32 distance pipeline: sqrt2 * sqrt(clip(1 - sim, 1e-6))."""
    sim = np.asarray(sim, dtype=np.float32)
    t = np.clip(np.float32(1.0) - sim, np.float32(1e-6), None).astype(np.float32)
    return (SQRT_2 * np.sqrt(t)).astype(np.float32)


def _select_winners(packed, a64, b64):
    """Pick per-row argmin-of-dist winners from top-8 packed candidates.

    packed: [PART, NSTRIP*TOPK] u32 for one core+direction; each value is
    (bits(sim+1) & QUANT_MASK) | column. a64, b64: fp64 descriptor sets
    (rows of S are a64 @ b64.T), used to (a) refine rows whose quantized
    candidate sims are too close to call, in exact reference fp32
    semantics, (b) recompute every winner's sim for the score.
    Returns (win_idx int64 [M], win_sim float32 [M]).
    """
    # [p, m, k] -> row r = m*PART + p
    T = packed.reshape(PART, NSTRIP, TOPK).transpose(1, 0, 2).reshape(M, TOPK)
    I = (T & np.uint32(IDX_MASK)).astype(np.int64)
    V = (T & np.uint32(QUANT_MASK)).view(np.float32) - np.float32(1.0)

    def pick(Vc, Ic):
        dist = _dist32(Vc)
        dmin = dist.min(axis=1, keepdims=True)
        tie = dist == dmin
        gi = np.where(tie, Ic, np.int64(1) << 40)
        return gi.min(axis=1)

    win_idx = pick(V, I)

    # Rows where several candidates sit within quantization distance of the
    # max: recompute their candidate sims in fp64 and redo the fp32 pipeline,
    # mirroring what the reference's own fp32 matmul would produce.
    vmax = V.max(axis=1, keepdims=True)
    near = (vmax - V) < REFINE_TH
    amb = np.flatnonzero(near.sum(axis=1) > 1)
    if os.environ.get("KERNEL_DEBUG"):
        print(f"[kernel] rows fp64-refined: {amb.size}/{len(V)}")
    if amb.size:
        sims64 = np.einsum("rd,rcd->rc", a64[amb], b64[I[amb]], optimize=True)
        win_idx[amb] = pick(sims64.astype(np.float32), I[amb])

    # Winner sims at full precision -> fp32 (reference-grade accuracy).
    win_sim = np.einsum("rd,rd->r", a64, b64[win_idx]).astype(np.float32)
    return win_idx, win_sim


def _match_batch_host(core_out, d0b, d1b):
    """Reproduce reference _match_batch from one core's candidate tensor."""
    d0_64 = d0b.astype(np.float64)
    d1_64 = d1b.astype(np.float64)
    half = NSTRIP * TOPK
    n_amin, sim_row = _select_winners(core_out[:, :half], d0_64, d1_64)
    m_amin, _ = _select_winners(core_out[:, half:], d1_64, d0_64)

    rng_m = np.arange(M, dtype=np.int64)
    mask = m_amin[n_amin] == rng_m

    dist_w = _dist32(sim_row)
    score = (np.float32(1.0) / (np.float32(1.0) + dist_w)).astype(np.float32)

    m0 = np.where(mask, n_amin, -1).astype(np.int32)
    ms0 = np.where(mask, score, np.float32(0.0)).astype(np.float32)

    m1 = np.full(N, -1, dtype=np.int32)
    ms1 = np.zeros(N, dtype=np.float32)
    sel = np.flatnonzero(mask)
    m1[n_amin[sel]] = sel.astype(np.int32)
    ms1[n_amin[sel]] = score[sel]
    return m0, ms0, m1, ms1


def kernel(kpts0, desc0, kpts1, desc1):
    desc0 = np.asarray(desc0, dtype=np.float32)
    desc1 = np.asarray(desc1, dtype=np.float32)
    assert desc0.shape == (B, M, D) and desc1.shape == (B, N, D)

    ab_all = prep_inputs(desc0, desc1)
    out = run_device(ab_all)
    kernel.last_results = out
    kernel.last_exec_time_ns = None

    m0 = np.empty((B, M), np.int32)
    ms0 = np.empty((B, M), np.float32)
    m1 = np.empty((B, N), np.int32)
    ms1 = np.empty((B, N), np.float32)
    for b in range(B):
        m0[b], ms0[b], m1[b], ms1[b] = _match_batch_host(
            out[b], desc0[b], desc1[b]
        )
    return m0, ms0, m1, ms1


# revision 12
# speedup vs baseline: 9.9671x; 1.1084x over previous
"""CycleMatcher (mutual-nearest-neighbor descriptor matching) on trn2.

Problem: B=4 pairs of L2-normalized descriptor sets d0,d1 [8192, 64].
dist = sqrt2*sqrt(clip(1 - d0@d1.T, 1e-6)); row/col argmins; mutual-NN
masking; scatter. dist is monotone-decreasing in sim = d0@d1.T, so argmin
dist == argmax sim (fp32 sqrt-rounding ties resolved on host in fp64).

The device kernel (~ a few ms) is dwarfed by the axon PJRT tunnel cost
(~60 ms/dispatch fixed + ~70 MB/s each way), so the design minimizes
bytes moved and dispatches:

- Sharding: 4 cores, one batch each; every core computes BOTH match
  directions (S = d0@d1.T row-argmax candidates and S.T row-argmax
  candidates), so each batch's descriptors are uploaded exactly once.
- Inputs are sent as ONE fp16 tensor per core [128, 8192]: partitions
  0-63 hold d0[b].T, 64-127 hold d1[b].T (8 MB total up vs 32 MB for the
  fp32 8-core layout). fp16 input rounding perturbs sims by ~1e-4, which
  the host-side fp64 refinement absorbs (see REFINE_TH).
- Outputs are 8 packed u32 PER ROW (2 MB total down vs 16 MB): the
  ScalarE PSUM drain computes sim+1.0 (maps sims into [1,2) where the
  IEEE fp32 bit pattern is monotone), DVE masks the low 13 mantissa bits
  and ORs in the column index (an iota), and a row-wide DVE max8 then
  yields the top-8 (quantized-sim, index) candidates in one value each.
  All 8192 packed row values are distinct (index bits), so max8 returns
  8 distinct columns, compared as positive fp32.
- Device program per direction: 64 row-strips x [64,128]^T @ [64,512]
  fp16 matmuls fill [128,2048] PSUM groups (double buffered); ScalarE
  drains+biases each group to SBUF; DVE packs and reduces.
- Dispatch: a module-cached jax.jit(shard_map(bass_exec)) instead of
  run_bass_kernel_spmd, which rebuilds + retraces the jit every call
  (~300 ms). The kernel writes every output element, so no donated
  zero output buffers are bound at all (run_bass_kernel_spmd uploads
  16 MB of zeros per call just to zero-init the outputs).

Host merges each row's 8 candidates, resolves fp16/quantization
near-ties exactly in reference fp32 semantics via fp64 recompute of the
candidate sims, recomputes every winner's sim in fp64->fp32 for the
score, then does the cheap mutual-NN match + scatter in numpy.
"""

import os
import sys

# Prefer whatever copy PYTHONPATH already provides (the axon sitecustomize
# puts /root/.axon_site/_ro/trn_rl_repo there); append fallbacks so kernel.py
# also works standalone without creating dual module identities.
for _p in ("/root/.axon_site/_ro/trn_rl_repo", "/opt/trn_rl_repo"):
    if _p not in sys.path:
        sys.path.append(_p)

import numpy as np

import concourse.bass as bass  # noqa: F401  (registers lowerings)
import concourse.mybir as mybir
import concourse.tile as tile
from concourse import bacc

B = 4
M = 8192
N = 8192
D = 64

NCORES = 4
PART = 128          # rows per strip (psum partitions)
NSTRIP = M // PART  # 64
MMN = 512           # matmul moving free dim (one psum bank, fp32 psum)
GRP = 2048          # psum group width
NG = N // GRP       # 4 groups per strip
TOPK = 8            # DVE max8 width = candidates per row
OUTW = 2 * NSTRIP * TOPK  # 1024 output cols per core (2 directions)

IDX_BITS = 13
IDX_MASK = (1 << IDX_BITS) - 1          # 0x1FFF
QUANT_MASK = 0xFFFFFFFF ^ IDX_MASK      # keep sign+exp+10 mantissa bits

SQRT_2 = np.float32(1.414213)

# Input wire format: "f16" (8 MB up) or "f8" (e4m3, 4 MB up). The PE
# accumulates either in fp32; coarser inputs only widen the band of rows
# the host must fp64-refine.
IN_DTYPE = os.environ.get("KERNEL_IN_DTYPE", "f8")

# Host fp64-refine threshold on candidate sims. Must cover 2x the worst
# device-vs-exact sim deviation: input rounding (f16: ~1e-4 rms, <5e-4
# tail; f8 e4m3: ~5.7e-3 rms, <2.5e-2 tail) + 13-bit packing quantization
# (<= 2^-10 ~ 9.8e-4, downward). f16: 4e-3 (~12% of rows refine);
# f8: 6e-2 (most rows refine — still a cheap numpy einsum).
REFINE_TH = np.float32(
    float(os.environ.get("KERNEL_REFINE_TH", "4e-3" if IN_DTYPE == "f16" else "6e-2"))
)

_cache = {}


def _build_program():
    nc = bacc.Bacc("TRN2", target_bir_lowering=False, debug=False)
    fin = mybir.dt.float16 if IN_DTYPE == "f16" else mybir.dt.float8e4
    f32 = mybir.dt.float32
    u32 = mybir.dt.uint32

    ab_d = nc.dram_tensor("ab", [2 * D, M], fin, kind="ExternalInput")
    out_d = nc.dram_tensor("out", [PART, OUTW], f32, kind="ExternalOutput")

    with tile.TileContext(nc) as tc:
        with (
            tc.tile_pool(name="inp", bufs=1) as inp,
            tc.tile_pool(name="outp", bufs=1) as outp,
            tc.tile_pool(name="ps", bufs=2, space="PSUM") as ps,
            tc.tile_pool(name="stage", bufs=4) as stage,
            tc.tile_pool(name="strip", bufs=2) as strippool,
        ):
            # Two [64, M] tiles (both base partition 0 — the PE requires
            # matmul operands to share their base partition) filled from
            # the halves of the single concatenated input tensor.
            a0 = inp.tile([D, M], fin)
            b0 = inp.tile([D, M], fin)
            nc.sync.dma_start(a0[:], ab_d.ap()[0:D, :])
            nc.scalar.dma_start(b0[:], ab_d.ap()[D:2 * D, :])

            # Global column index, identical on every partition.
            it = inp.tile([PART, M], u32)
            nc.gpsimd.iota(it[:], [[1, M]], channel_multiplier=0)

            top8 = outp.tile([PART, OUTW], f32)

            for d in range(2):
                at = a0 if d == 0 else b0
                bt = b0 if d == 0 else a0
                for m in range(NSTRIP):
                    lhsT = at[:, m * PART:(m + 1) * PART]  # [64, 128] stationary
                    pk = strippool.tile([PART, M], u32)
                    pkf = pk[:].bitcast(f32)
                    for g in range(NG):
                        pt = ps.tile([PART, GRP], f32)
                        for j in range(GRP // MMN):
                            n0 = g * GRP + j * MMN
                            nc.tensor.matmul(
                                pt[:, j * MMN:(j + 1) * MMN],
                                lhsT,
                                bt[:, n0:n0 + MMN],
                                start=True,
                                stop=True,
                            )
                        # ScalarE drain with +1.0 bias: sims -> [1, 2) where
                        # the fp32 bit pattern is monotone in the value.
                        st = stage.tile([PART, GRP], f32)
                        nc.scalar.add(st[:], pt[:], 1.0)
                        gsl = pk[:, g * GRP:(g + 1) * GRP]
                        # quantize (drop low 13 mantissa bits) ...
                        nc.vector.tensor_scalar(
                            gsl,
                            st[:].bitcast(u32),
                            QUANT_MASK,
                            None,
                            mybir.AluOpType.bitwise_and,
                        )
                        # ... and OR in the global column index.
                        nc.vector.tensor_tensor(
                            gsl,
                            gsl,
                            it[:, g * GRP:(g + 1) * GRP],
                            mybir.AluOpType.bitwise_or,
                        )
                    c0 = (d * NSTRIP + m) * TOPK
                    nc.vector.max(out=top8[:, c0:c0 + TOPK], in_=pkf)

            nc.sync.dma_start(out_d.ap(), top8[:])

    nc.compile()
    return nc


def _get_dispatcher():
    """Build (once) the jitted shard_map dispatch for the bass program.

    Replicates concourse.bass2jax.run_bass_via_pjrt but (a) caches the
    jitted callable (run_bass_via_pjrt re-creates and re-traces it every
    call) and (b) binds NO output operands: the kernel writes every output
    element, so the custom-call results need no zero-init donation.
    """
    if "disp" in _cache:
        return _cache["disp"]

    import jax
    from jax.experimental.shard_map import shard_map
    from jax.sharding import Mesh, PartitionSpec

    from concourse.bass2jax import (
        _bass_exec_p,
        install_neuronx_cc_hook,
        partition_id_tensor,
    )

    nc = _build_program()
    install_neuronx_cc_hook()

    partition_name = nc.partition_id_tensor.name if nc.partition_id_tensor else None
    in_names = []
    out_names = []
    out_avals = []
    for alloc in nc.m.functions[0].allocations:
        if not isinstance(alloc, mybir.MemoryLocationSet):
            continue
        name = alloc.memorylocations[0].name
        if alloc.kind == "ExternalInput":
            if name != partition_name:
                in_names.append(name)
        elif alloc.kind == "ExternalOutput":
            shape = tuple(alloc.tensor_shape)
            dtype = mybir.dt.np(alloc.dtype)
            out_names.append(name)
            out_avals.append(jax.core.ShapedArray(shape, dtype))
    all_names = tuple(in_names)
    if partition_name is not None:
        all_names = all_names + (partition_name,)

    def _body(*args):
        operands = list(args)
        if partition_name is not None:
            operands.append(partition_id_tensor())
        outs = _bass_exec_p.bind(
            *operands,
            out_avals=tuple(out_avals),
            in_names=all_names,
            out_names=tuple(out_names),
            lowering_input_output_aliases=(),
            sim_require_finite=True,
            sim_require_nnan=True,
            nc=nc,
        )
        return tuple(outs)

    devices = jax.devices()[:NCORES]
    mesh = Mesh(np.asarray(devices), ("core",))
    spec = PartitionSpec("core")
    sharded = jax.jit(
        shard_map(
            _body,
            mesh=mesh,
            in_specs=(spec,) * len(in_names),
            out_specs=(spec,) * len(out_names),
            check_rep=False,
        ),
        keep_unused=True,
    )

    _cache["disp"] = sharded
    return _cache["disp"]


def prep_inputs(desc0, desc1):
    """fp32 [B, M, D] descriptor pair -> concat f16/f8 device input [B*128, M]."""
    np_in = mybir.dt.np(mybir.dt.float16 if IN_DTYPE == "f16" else mybir.dt.float8e4)
    ab = np.empty((B, 2 * D, M), np_in)
    ab[:, :D] = desc0.transpose(0, 2, 1)
    ab[:, D:] = desc1.transpose(0, 2, 1)
    return ab.reshape(B * 2 * D, M)


def run_device(ab_all):
    """One device dispatch: [512, 8192] f8/f16 in -> [4, 128, 1024] u32 out."""
    sharded = _get_dispatcher()
    (out,) = sharded(ab_all)
    return np.asarray(out).view(np.uint32).reshape(NCORES, PART, OUTW)


def _dist32(sim):
    """Reference fp32 distance pipeline: sqrt2 * sqrt(clip(1 - sim, 1e-6))."""
    sim = np.asarray(sim, dtype=np.float32)
    t = np.clip(np.float32(1.0) - sim, np.float32(1e-6), None).astype(np.float32)
    return (SQRT_2 * np.sqrt(t)).astype(np.float32)


def _select_winners(packed, a64, b64):
    """Pick per-row argmin-of-dist winners from top-8 packed candidates.

    packed: [PART, NSTRIP*TOPK] u32 for one core+direction; each value is
    (bits(sim+1) & QUANT_MASK) | column. a64, b64: fp64 descriptor sets
    (rows of S are a64 @ b64.T), used to (a) refine rows whose quantized
    candidate sims are too close to call, in exact reference fp32
    semantics, (b) recompute every winner's sim for the score.
    Returns (win_idx int64 [M], win_sim float32 [M]).
    """
    # [p, m, k] -> row r = m*PART + p
    T = packed.reshape(PART, NSTRIP, TOPK).transpose(1, 0, 2).reshape(M, TOPK)
    I = (T & np.uint32(IDX_MASK)).astype(np.int64)
    V = (T & np.uint32(QUANT_MASK)).view(np.float32) - np.float32(1.0)

    def pick(Vc, Ic):
        dist = _dist32(Vc)
        dmin = dist.min(axis=1, keepdims=True)
        tie = dist == dmin
        gi = np.where(tie, Ic, np.int64(1) << 40)
        return gi.min(axis=1)

    win_idx = pick(V, I)

    # Rows where several candidates sit within quantization distance of the
    # max: recompute their candidate sims in fp64 and redo the fp32 pipeline,
    # mirroring what the reference's own fp32 matmul would produce.
    vmax = V.max(axis=1, keepdims=True)
    near = (vmax - V) < REFINE_TH
    amb = np.flatnonzero(near.sum(axis=1) > 1)
    if os.environ.get("KERNEL_DEBUG"):
        print(f"[kernel] rows fp64-refined: {amb.size}/{len(V)}")
    if amb.size:
        sims64 = np.einsum("rd,rcd->rc", a64[amb], b64[I[amb]], optimize=True)
        win_idx[amb] = pick(sims64.astype(np.float32), I[amb])

    # Winner sims at full precision -> fp32 (reference-grade accuracy).
    win_sim = np.einsum("rd,rd->r", a64, b64[win_idx]).astype(np.float32)
    return win_idx, win_sim


def _match_batch_host(core_out, d0b, d1b):
    """Reproduce reference _match_batch from one core's candidate tensor."""
    d0_64 = d0b.astype(np.float64)
    d1_64 = d1b.astype(np.float64)
    half = NSTRIP * TOPK
    n_amin, sim_row = _select_winners(core_out[:, :half], d0_64, d1_64)
    m_amin, _ = _select_winners(core_out[:, half:], d1_64, d0_64)

    rng_m = np.arange(M, dtype=np.int64)
    mask = m_amin[n_amin] == rng_m

    dist_w = _dist32(sim_row)
    score = (np.float32(1.0) / (np.float32(1.0) + dist_w)).astype(np.float32)

    m0 = np.where(mask, n_amin, -1).astype(np.int32)
    ms0 = np.where(mask, score, np.float32(0.0)).astype(np.float32)

    m1 = np.full(N, -1, dtype=np.int32)
    ms1 = np.zeros(N, dtype=np.float32)
    sel = np.flatnonzero(mask)
    m1[n_amin[sel]] = sel.astype(np.int32)
    ms1[n_amin[sel]] = score[sel]
    return m0, ms0, m1, ms1


def kernel(kpts0, desc0, kpts1, desc1):
    desc0 = np.asarray(desc0, dtype=np.float32)
    desc1 = np.asarray(desc1, dtype=np.float32)
    assert desc0.shape == (B, M, D) and desc1.shape == (B, N, D)

    ab_all = prep_inputs(desc0, desc1)
    out = run_device(ab_all)
    kernel.last_results = out
    kernel.last_exec_time_ns = None

    m0 = np.empty((B, M), np.int32)
    ms0 = np.empty((B, M), np.float32)
    m1 = np.empty((B, N), np.int32)
    ms1 = np.empty((B, N), np.float32)
    for b in range(B):
        m0[b], ms0[b], m1[b], ms1[b] = _match_batch_host(
            out[b], desc0[b], desc1[b]
        )
    return m0, ms0, m1, ms1
